# revision 1
# baseline (speedup 1.0000x reference)
"""ChunkedSparseAttention Trainium2 kernel.

Problem: B=2, S=4096, D=1024, CHUNK=64. Per chunk i:
  local  = softmax(Qi @ Ki^T / 32) @ Vi            (own 64 keys)
  cross  = softmax(Qi @ K[:64i]^T / 32) @ V[:64i]  (prefix keys)
  out_i  = local                   if i == 0
         = 0.9 * local + 0.1 * cross otherwise

Distribution: 8 cores, data-parallel over batch (4 cores/batch), with each
core taking one quad-chunk "group" (256 queries) from each of 4 classes
({0-3},{4-7},{8-11},{12-15}) so the triangular prefix work is balanced.
All cores run ONE SPMD NEFF: per-class kb loops are padded to the class max
and masked via a per-core bias table (exp(s/32 + bias), bias=-1e9 kills
padded key blocks). Per-core data differences are handled by host-side
gathers (queries, boundary keys/values, bias, blend coefficients).

On-chip layout ("S^T layout"): scores are computed keys-on-partitions,
S^T[k,q] = sum_d K^T[d,k] Q^T[d,q], so exp(S^T) is directly the lhsT of the
P@V matmul (no transposes on the critical path) and softmax denominators
come from a ones-column matmul. K^T/Q^T are pre-transposed on the host.
Matmuls run in float32r (full PE rate at N>=256, ~1.6e-4 rel err).
"""
import sys

for _p in ("/opt/trn_rl_repo", "/root/.axon_site/_ro/trn_rl_repo"):
    if _p not in sys.path:
        sys.path.insert(0, _p)

import numpy as np

import concourse.bass as bass
import concourse.mybir as mybir
import concourse.tile as tile
from concourse import bacc
from concourse.bass_utils import run_bass_kernel_spmd

F32 = mybir.dt.float32
F32R = mybir.dt.float32r
AF = mybir.ActivationFunctionType
SCALE = 1.0 / 32.0  # 1/sqrt(D)
NEG = -1e9


class Cfg:
    def __init__(self, S, classes):
        self.S = S
        self.D = 1024
        self.classes = classes            # list of 4 lists of group indices
        self.n_slot = len(classes)
        self.M = [2 * max(c) for c in classes]   # padded full-kb count per slot
        self.M = [max(m, 2) for m in self.M]
        self.maxM = max(self.M)
        self.GQ = 256                      # queries per group (4 chunks)
        self.NQ = self.n_slot * self.GQ    # queries per core
        self.n_dblk = self.D // 128
        self.cores_per_batch = len(classes[0])
        self.n_cores = 2 * self.cores_per_batch


FULL = Cfg(4096, [[0, 1, 2, 3], [4, 5, 6, 7], [8, 9, 10, 11], [12, 13, 14, 15]])
MINI = Cfg(1024, [[0], [1], [2], [3]])


def build_nc(cfg: Cfg):
    S, D = cfg.S, cfg.D
    NDB = cfg.n_dblk
    nc = bacc.Bacc("TRN2", target_bir_lowering=False, debug=False)

    kt_in = nc.dram_tensor("kt_in", [D, S], F32, kind="ExternalInput")
    qt_in = nc.dram_tensor("qt_in", [D, cfg.NQ], F32, kind="ExternalInput")
    kbt_in = nc.dram_tensor("kbt_in", [D, cfg.NQ], F32, kind="ExternalInput")
    v_in = nc.dram_tensor("v_in", [S, D], F32, kind="ExternalInput")
    vb_in = nc.dram_tensor("vb_in", [cfg.NQ, D], F32, kind="ExternalInput")
    bias_in = nc.dram_tensor("bias_in", [cfg.n_slot, 128, cfg.maxM], F32,
                             kind="ExternalInput")
    blend_in = nc.dram_tensor("blend_in", [cfg.n_slot, 128, 4], F32,
                              kind="ExternalInput")
    out_t = nc.dram_tensor("out_core", [cfg.NQ, D], F32, kind="ExternalOutput")
    dbg = getattr(cfg, "debug", False)
    if dbg:
        dbg_s = nc.dram_tensor("dbg_sums", [cfg.n_slot, 2, 128, 4], F32,
                               kind="ExternalOutput")
        dbg_o = nc.dram_tensor("dbg_o", [cfg.n_slot, 2, 128, D], F32,
                               kind="ExternalOutput")
    ones_dr = nc.inline_tensor(np.ones((128, 2), np.float32), "ones_c")

    with tile.TileContext(nc) as tc:
        with (
            tc.tile_pool(name="const", bufs=1) as cpool,
            tc.tile_pool(name="kt", bufs=1) as ktp,
            tc.tile_pool(name="qt", bufs=2) as qtp,
            tc.tile_pool(name="kbt", bufs=1) as kbtp,
            tc.tile_pool(name="vb", bufs=1) as vbp,
            tc.tile_pool(name="vsrc", bufs=3) as vsp,
            tc.tile_pool(name="vcast", bufs=3) as vcp,
            tc.tile_pool(name="et", bufs=4) as etp,
            tc.tile_pool(name="eb", bufs=3) as ebp,
            tc.tile_pool(name="bias", bufs=3) as biasp,
            tc.tile_pool(name="blend", bufs=2) as blp,
            tc.tile_pool(name="vec", bufs=10) as vecp,
            tc.tile_pool(name="outst", bufs=3) as outp,
            tc.tile_pool(name="poc", bufs=2, space="PSUM") as poc,
            tc.tile_pool(name="pst", bufs=2, space="PSUM") as pst,
            tc.tile_pool(name="psm", bufs=2, space="PSUM") as psm,
        ):
            ones_t = cpool.tile([128, 2], F32R)
            nc.gpsimd.dma_start(ones_t[:], ones_dr[:])
            ones_f32 = ones_t[:].bitcast(F32)

            # resident K^T, DMA-cast to f32r: [128(d), NDB, S]. Split along S
            # so early score matmuls only wait on the first column ranges
            # instead of the whole 16MB transfer.
            kt = ktp.tile([128, NDB, S], F32R)
            for i in range(8):
                c0, c1 = i * (S // 8), (i + 1) * (S // 8)
                nc.gpsimd.dma_start(
                    kt[:, :, c0:c1],
                    kt_in[:, c0:c1].rearrange("(db p) s -> p db s", p=128))

            for j in range(cfg.n_slot):
                Mj = cfg.M[j]
                qcol = j * cfg.GQ

                # per-slot Q^T, Kb^T (DMA-cast from host-transposed gathers)
                qt = qtp.tile([128, NDB, cfg.GQ], F32R)
                nc.gpsimd.dma_start(
                    qt[:], qt_in[:, qcol:qcol + cfg.GQ]
                    .rearrange("(db p) q -> p db q", p=128))
                kbt = kbtp.tile([128, NDB, cfg.GQ], F32R)
                nc.gpsimd.dma_start(
                    kbt[:], kbt_in[:, qcol:qcol + cfg.GQ]
                    .rearrange("(db p) q -> p db q", p=128))
                vb = vbp.tile([128, 2, D], F32R)
                nc.gpsimd.dma_start(
                    vb[:], vb_in[qcol:qcol + cfg.GQ, :]
                    .rearrange("(c p) d -> p c d", p=128))
                blend = blp.tile([128, 4], F32)
                nc.sync.dma_start(blend[:], blend_in[j])
                bias_slot = biasp.tile([128, cfg.maxM], F32)
                nc.sync.dma_start(bias_slot[:], bias_in[j])

                oc = [poc.tile([128, D], F32, tag="oc", name=f"oc{s}_{j}")
                      for s in range(2)]
                # one PSUM bank per accumulation chain: a second chain's
                # start=True in the same bank clobbers the first chain's
                # has_written state, so each sub's running sums gets its own
                # bank-padded tile.
                sums_c = [psm.tile([128, 2], F32, tag="sums", name=f"sc{s}_{j}")
                          for s in range(2)]

                # ---- full-kb loop (software-pipelined: QK(kb) then PV(kb-1))
                ets = {}
                vts = {}

                def emit_qk(kb):
                    vsrc = vsp.tile([128, D], F32)
                    nc.sync.dma_start(vsrc[:], v_in[kb * 128:(kb + 1) * 128, :])
                    vt = vcp.tile([128, D], F32R)
                    nc.vector.tensor_copy(vt[:], vsrc[:])
                    vts[kb] = vt
                    st = pst.tile([128, cfg.GQ], F32, tag="st")
                    for db in range(NDB):
                        nc.tensor.matmul(
                            st[:], kt[:, db, kb * 128:(kb + 1) * 128],
                            qt[:, db, :], start=(db == 0), stop=(db == NDB - 1))
                    et = etp.tile([128, cfg.GQ], F32R)
                    nc.scalar.activation(et[:], st[:], AF.Exp,
                                         bias=bias_slot[:, kb:kb + 1],
                                         scale=SCALE)
                    ets[kb] = et

                def emit_pv(kb):
                    et, vt = ets.pop(kb), vts.pop(kb)
                    for sub in range(2):
                        lhs = et[:, sub * 128:(sub + 1) * 128]
                        for dh in range(2):
                            nc.tensor.matmul(
                                oc[sub][:, dh * 512:(dh + 1) * 512], lhs,
                                vt[:, dh * 512:(dh + 1) * 512],
                                start=(kb == 0), stop=False)
                        nc.tensor.matmul(sums_c[sub][:], lhs,
                                         ones_t[:], start=(kb == 0), stop=False)

                for kb in range(Mj + 1):
                    if kb < Mj:
                        emit_qk(kb)
                    if kb >= 1:
                        emit_pv(kb - 1)

                # ---- boundary blocks b0/b1 (the group's own 256 keys)
                ebs = []
                for blk in range(2):
                    st = pst.tile([128, cfg.GQ], F32, tag="st")
                    for db in range(NDB):
                        nc.tensor.matmul(
                            st[:], kbt[:, db, blk * 128:(blk + 1) * 128],
                            qt[:, db, :], start=(db == 0), stop=(db == NDB - 1))
                    eb = ebp.tile([128, cfg.GQ], F32R)
                    nc.scalar.activation(eb[:], st[:], AF.Exp, scale=SCALE)
                    ebs.append(eb)
                eb0, eb1 = ebs

                # cross pieces within the boundary:
                # q1 (chunk 4g+1) <- first half of b0; dst partitions 64:128 -> fp32
                for dh in range(2):
                    nc.tensor.matmul(
                        oc[0][64:128, dh * 512:(dh + 1) * 512],
                        eb0[0:64, 64:128].bitcast(F32),
                        vb[0:64, 0, dh * 512:(dh + 1) * 512].bitcast(F32),
                        start=False, stop=(dh == 1))
                nc.tensor.matmul(sums_c[0][64:128, :],
                                 eb0[0:64, 64:128].bitcast(F32),
                                 ones_f32[0:64, :], start=False, stop=True)
                # q2,q3 <- all of b0; dst partitions 0:128 -> f32r
                for dh in range(2):
                    nc.tensor.matmul(
                        oc[1][:, dh * 512:(dh + 1) * 512],
                        eb0[:, 128:256], vb[:, 0, dh * 512:(dh + 1) * 512],
                        start=False, stop=False)
                nc.tensor.matmul(sums_c[1][:], eb0[:, 128:256], ones_t[:],
                                 start=False, stop=False)
                # q3 <- first half of b1; dst partitions 64:128 -> fp32
                for dh in range(2):
                    nc.tensor.matmul(
                        oc[1][64:128, dh * 512:(dh + 1) * 512],
                        eb1[0:64, 192:256].bitcast(F32),
                        vb[0:64, 1, dh * 512:(dh + 1) * 512].bitcast(F32),
                        start=False, stop=(dh == 1))
                nc.tensor.matmul(sums_c[1][64:128, :],
                                 eb1[0:64, 192:256].bitcast(F32),
                                 ones_f32[0:64, :], start=False, stop=True)

                # ---- flush cross, then local per sub (L reuses oc pool slots)
                sums_l = pst.tile([128, 4], F32, tag="st", name=f"sl_{j}")
                for sub in range(2):
                    eb = ebs[sub]
                    # cross normalization * alpha
                    scm = vecp.tile([128, 1], F32, tag="v")
                    nc.vector.tensor_scalar_max(
                        scm[:], sums_c[sub][:, 0:1], 1e-30)
                    rc = vecp.tile([128, 1], F32, tag="v")
                    nc.vector.reciprocal(rc[:], scm[:])
                    rc2 = vecp.tile([128, 1], F32, tag="v")
                    nc.vector.tensor_mul(rc2[:], rc[:],
                                         blend[:, 2 * sub + 1:2 * sub + 2])
                    cs = outp.tile([128, D], F32, tag="out")
                    nc.scalar.activation(cs[:], oc[sub][:], AF.Copy,
                                         scale=rc2[:])
                    if dbg:
                        dso = outp.tile([128, D], F32, tag="out")
                        nc.vector.tensor_copy(dso[:], oc[sub][:])
                        nc.sync.dma_start(dbg_o[j, sub], dso[:])
                        dss = vecp.tile([128, 2], F32, tag="dv", name="dss")
                        nc.vector.tensor_copy(dss[:], sums_c[sub][:])
                        nc.sync.dma_start(dbg_s[j, 0, :, 2 * sub:2 * sub + 2],
                                          dss[:])

                    # local attention for the two chunks of this sub
                    L = poc.tile([128, D], F32, tag="oc")
                    for dh in range(2):
                        nc.tensor.matmul(  # even chunk: partitions 0:64, f32r
                            L[0:64, dh * 512:(dh + 1) * 512],
                            eb[0:64, sub * 128:sub * 128 + 64],
                            vb[0:64, sub, dh * 512:(dh + 1) * 512],
                            start=True, stop=True)
                        nc.tensor.matmul(  # odd chunk: partitions 64:128, fp32
                            L[64:128, dh * 512:(dh + 1) * 512],
                            eb[64:128, sub * 128 + 64:sub * 128 + 128]
                            .bitcast(F32),
                            vb[64:128, sub, dh * 512:(dh + 1) * 512]
                            .bitcast(F32),
                            start=True, stop=True)
                    nc.tensor.matmul(sums_l[0:64, 2 * sub:2 * sub + 2],
                                     eb[0:64, sub * 128:sub * 128 + 64],
                                     ones_t[0:64, :], start=True, stop=True)
                    nc.tensor.matmul(sums_l[64:128, 2 * sub:2 * sub + 2],
                                     eb[64:128, sub * 128 + 64:sub * 128 + 128]
                                     .bitcast(F32),
                                     ones_f32[64:128, :], start=True, stop=True)

                    if dbg and sub == 1:
                        dsl = vecp.tile([128, 4], F32, tag="dv", name="dsl")
                        nc.vector.tensor_copy(dsl[:], sums_l[:])
                        nc.sync.dma_start(dbg_s[j, 1], dsl[:])
                    slm = vecp.tile([128, 1], F32, tag="v")
                    nc.vector.tensor_scalar_max(
                        slm[:], sums_l[:, 2 * sub:2 * sub + 1], 1e-30)
                    rl = vecp.tile([128, 1], F32, tag="v")
                    nc.vector.reciprocal(rl[:], slm[:])
                    rl2 = vecp.tile([128, 1], F32, tag="v")
                    nc.vector.tensor_mul(rl2[:], rl[:],
                                         blend[:, 2 * sub:2 * sub + 1])
                    lt = outp.tile([128, D], F32, tag="out")
                    nc.vector.tensor_scalar_mul(lt[:], L[:], rl2[:])
                    fin = outp.tile([128, D], F32, tag="out")
                    nc.vector.tensor_add(fin[:], lt[:], cs[:])
                    row = (2 * j + sub) * 128
                    nc.sync.dma_start(out_t[row:row + 128, :], fin[:])
    nc.compile()
    return nc


def _host_inputs(cfg: Cfg, query, key, value):
    """Build the 2*cores_per_batch per-core input maps."""
    in_maps = []
    for core in range(cfg.n_cores):
        b = core // cfg.cores_per_batch
        qt_idx = core % cfg.cores_per_batch
        groups = [cls[qt_idx] for cls in cfg.classes]
        kt_core = np.ascontiguousarray(key[b].T)
        q_rows = np.concatenate(
            [query[b, g * cfg.GQ:(g + 1) * cfg.GQ] for g in groups])
        kb_rows = np.concatenate(
            [key[b, g * cfg.GQ:(g + 1) * cfg.GQ] for g in groups])
        vb_rows = np.concatenate(
            [value[b, g * cfg.GQ:(g + 1) * cfg.GQ] for g in groups])
        bias = np.zeros((cfg.n_slot, 128, cfg.maxM), np.float32)
        blend = np.zeros((cfg.n_slot, 128, 4), np.float32)
        for j, g in enumerate(groups):
            bias[j, :, 2 * g:] = NEG
            for sub in range(2):
                for half in range(2):
                    chunk = 4 * g + 2 * sub + half
                    sl = slice(half * 64, half * 64 + 64)
                    blend[j, sl, 2 * sub] = 1.0 if chunk == 0 else 0.9
                    blend[j, sl, 2 * sub + 1] = 0.0 if chunk == 0 else 0.1
        in_maps.append({
            "kt_in": kt_core,
            "qt_in": np.ascontiguousarray(q_rows.T),
            "kbt_in": np.ascontiguousarray(kb_rows.T),
            "v_in": np.ascontiguousarray(value[b]),
            "vb_in": vb_rows,
            "bias_in": bias,
            "blend_in": blend,
        })
    return in_maps


def _scatter_output(cfg: Cfg, results, B):
    out = np.empty((B, cfg.S, cfg.D), np.float32)
    for core in range(cfg.n_cores):
        b = core // cfg.cores_per_batch
        qt_idx = core % cfg.cores_per_batch
        groups = [cls[qt_idx] for cls in cfg.classes]
        oc = results[core]["out_core"]
        for j, g in enumerate(groups):
            out[b, g * cfg.GQ:(g + 1) * cfg.GQ] = oc[j * cfg.GQ:(j + 1) * cfg.GQ]
    return out


_nc_cache = {}


def run(cfg: Cfg, query, key, value, trace=False, trace_kwargs=None):
    ck = (cfg.S, getattr(cfg, "debug", False))
    if ck not in _nc_cache:
        _nc_cache[ck] = build_nc(cfg)
    nc = _nc_cache[ck]
    in_maps = _host_inputs(cfg, query, key, value)
    kw = {}
    if trace:
        kw = dict(trace=True, trace_cores=list(range(cfg.n_cores)),
                  **(trace_kwargs or {}))
    res = run_bass_kernel_spmd(nc, in_maps, core_ids=list(range(cfg.n_cores)),
                               **kw)
    out = _scatter_output(cfg, res.results, query.shape[0])
    return out, res


def kernel(query, key, value):
    query = np.asarray(query, np.float32)
    key = np.asarray(key, np.float32)
    value = np.asarray(value, np.float32)
    out, _ = run(FULL, query, key, value)
    return out



# revision 6
# speedup vs baseline: 2.8018x; 2.8018x over previous
"""ChunkedSparseAttention Trainium2 kernel (fp8 DoubleRow redesign).

Problem: B=2, S=4096, D=1024, CHUNK=64. Per chunk i:
  local  = softmax(Qi @ Ki^T / 32) @ Vi            (own 64 keys)
  cross  = softmax(Qi @ K[:64i]^T / 32) @ V[:64i]  (prefix keys)
  out_i  = local                     if i == 0
         = 0.9 * local + 0.1 * cross otherwise

Distribution: 8 cores, data-parallel over batch (4 cores/batch); each core
takes one quad-chunk group (256 queries) from each of 4 classes
({0-3},{4-7},{8-11},{12-15}) so triangular prefix work is balanced. One SPMD
NEFF; per-class prefix loops padded to the class max and masked via a
per-core bias table fed to the exp activation.

Numerics/speed design (cost-model driven):
- All cross-attention matmuls (97% of FLOPs, alpha=0.1 so fp8 noise is 10x
  attenuated) run in fp8e4 with MatmulPerfMode.DoubleRow: each instruction
  contracts 2x128 rows at 0.5 cycles/output-column (4x the f32r rate).
  Scores keep keys on partitions (S^T layout) so exp(S^T) is directly the
  lhsT of the P@V matmul.
- exp is applied per key-block PAIR (one activation over a [128,2,256] PSUM
  tile = one full bank; single start=True initializes the whole bank).
- Scores are shifted by -2 inside the exp (fp8 range), which cancels in the
  softmax normalization.
- The group's own 256-key block (boundary) gets 3-term two-level fp8 scores
  (hi*hi + hi*lo + lo*hi with lo = fp8 residual) so the precision-sensitive
  local diagonal is accurate to ~2e-3; the strictly-lower wedge (cross) is
  exp'd to bf16 with partition-masked biases and multiplied against bf16 V.
- Local 64x64 diagonal: exp'd into a block-diagonal bf16 lhsT, one matmul
  per 512-wide D half against bf16 V.
- Output is written f16 and upcast on the host.
"""
import sys

for _p in ("/opt/trn_rl_repo", "/root/.axon_site/_ro/trn_rl_repo"):
    if _p not in sys.path:
        sys.path.insert(0, _p)

import numpy as np
import ml_dtypes

import concourse.bass as bass
import concourse.mybir as mybir
import concourse.tile as tile
from concourse import bacc
from concourse.bass_utils import run_bass_kernel_spmd

F32 = mybir.dt.float32
F8 = mybir.dt.float8e4
BF16 = mybir.dt.bfloat16
F16 = mybir.dt.float16
AF = mybir.ActivationFunctionType
DR = mybir.MatmulPerfMode.DoubleRow

FP8 = ml_dtypes.float8_e4m3
BF = ml_dtypes.bfloat16

SCALE = 1.0 / 32.0  # 1/sqrt(D)
SHIFT = -2.0        # exp shift, cancels in normalization
NEG = -1e9


class Cfg:
    def __init__(self, S, classes):
        self.S = S
        self.D = 1024
        self.classes = classes              # 4 lists of group indices
        self.n_slot = len(classes)
        self.PJ = [max(c) for c in classes]  # padded prefix pair count/slot
        self.maxP = max(self.PJ)
        self.SPRE = 2 * self.maxP * 128      # prefix keys covered by kt8/v8
        self.GQ = 256
        self.NQ = self.n_slot * self.GQ
        self.cores_per_batch = len(classes[0])
        self.n_cores = 2 * self.cores_per_batch


FULL = Cfg(4096, [[0, 1, 2, 3], [4, 5, 6, 7], [8, 9, 10, 11], [12, 13, 14, 15]])
MINI = Cfg(1024, [[0], [1], [2], [3]])


def build_nc(cfg: Cfg):
    D = cfg.D
    NSL = cfg.n_slot
    maxP = max(cfg.maxP, 1)
    nc = bacc.Bacc("TRN2", target_bir_lowering=False, debug=False)

    kt8_in = nc.dram_tensor("kt8_in", [128, 4, 2, max(cfg.SPRE, 256)], F8,
                            kind="ExternalInput")
    v8_in = nc.dram_tensor("v8_in", [128, max(cfg.maxP, 1), 2, D], F8,
                           kind="ExternalInput")
    # per-slot gather slab: kinds = (q_hi, q_lo, kb_hi, kb_lo)
    qslab_in = nc.dram_tensor("qslab_in", [128, NSL, 4, 4, 2, 256], F8,
                              kind="ExternalInput")
    vb16_in = nc.dram_tensor("vb16_in", [128, NSL, 2, D], BF16,
                             kind="ExternalInput")
    biasp_in = nc.dram_tensor("biasp_in", [128, NSL, maxP], F32,
                              kind="ExternalInput")
    blend_in = nc.dram_tensor("blend_in", [128, NSL, 4], F32,
                              kind="ExternalInput")
    out_t = nc.dram_tensor("out_core", [cfg.NQ, D], F16, kind="ExternalOutput")

    ones8_dr = nc.inline_tensor(np.ones((128, 2, 2), FP8), "ones8_c")
    ones16_dr = nc.inline_tensor(np.ones((128, 2), BF), "ones16_c")
    bb = np.full((128, 2), NEG, np.float32)
    bb[0:64, 0] = SHIFT
    bb[:, 1] = SHIFT
    biasb_dr = nc.inline_tensor(bb, "biasb_c")

    with tile.TileContext(nc) as tc:
        with (
            tc.tile_pool(name="const", bufs=1) as cpool,
            tc.tile_pool(name="kt8", bufs=1) as ktp,
            tc.tile_pool(name="v8", bufs=1) as v8p,
            tc.tile_pool(name="qslab", bufs=1) as qsp,
            tc.tile_pool(name="vb16", bufs=1) as vbp,
            tc.tile_pool(name="et", bufs=3) as etp,
            tc.tile_pool(name="ew", bufs=2) as ewp,
            tc.tile_pool(name="lh", bufs=2) as lhp,
            tc.tile_pool(name="vec", bufs=10) as vecp,
            tc.tile_pool(name="outst", bufs=4) as outp,
            tc.tile_pool(name="poc", bufs=2, space="PSUM") as poc,
            tc.tile_pool(name="pst", bufs=2, space="PSUM") as pst,
            tc.tile_pool(name="psm", bufs=2, space="PSUM") as psm,
        ):
            ones8 = cpool.tile([128, 2, 2], F8)
            nc.sync.dma_start(ones8[:], ones8_dr[:])
            ones16 = cpool.tile([128, 2], BF16)
            nc.sync.dma_start(ones16[:], ones16_dr[:])
            biasb = cpool.tile([128, 2], F32)
            nc.sync.dma_start(biasb[:], biasb_dr[:])
            biasp = cpool.tile([128, NSL, maxP], F32)
            nc.sync.dma_start(biasp[:], biasp_in[:])
            blend = cpool.tile([128, NSL, 4], F32)
            nc.sync.dma_start(blend[:], blend_in[:])

            kt8 = ktp.tile([128, 4, 2, max(cfg.SPRE, 256)], F8)
            v8 = v8p.tile([128, max(cfg.maxP, 1), 2, D], F8)
            qslab = qsp.tile([128, NSL, 4, 4, 2, 256], F8)
            vb16 = vbp.tile([128, NSL, 2, D], BF16)

            # interleave input DMAs roughly in order of first use
            ksp = max(cfg.SPRE, 256) // 4
            vsp = (max(cfg.maxP, 1) + 3) // 4
            for j in range(NSL):
                nc.sync.dma_start(qslab[:, j], qslab_in[:, j])
                c0, c1 = j * ksp, (j + 1) * ksp
                nc.sync.dma_start(kt8[:, :, :, c0:c1], kt8_in[:, :, :, c0:c1])
                u0, u1 = j * vsp, min((j + 1) * vsp, max(cfg.maxP, 1))
                if u0 < u1:
                    nc.sync.dma_start(v8[:, u0:u1], v8_in[:, u0:u1])
                nc.sync.dma_start(vb16[:, j], vb16_in[:, j])

            for j in range(NSL):
                PJn = cfg.PJ[j]
                qt8 = qslab[:, j, 0]
                qlo = qslab[:, j, 1]
                kb8 = qslab[:, j, 2]
                klo = qslab[:, j, 3]

                oc = [poc.tile([128, D], F32, tag="oc", name=f"oc{s}_{j}")
                      for s in range(2)]
                sums = [psm.tile([128, 2], F32, tag="sums", name=f"sc{s}_{j}")
                        for s in range(2)]

                # ---- prefix pairs (software-pipelined, depth 2)
                ets = {}

                def emit_qk(u):
                    st = pst.tile([128, 2, 256], F32, tag="st")
                    for i in range(2):
                        kb = 2 * u + i
                        for dp in range(4):
                            nc.tensor.matmul(
                                st[:, i, :],
                                kt8[:, dp, 0:2, kb * 128:(kb + 1) * 128],
                                qt8[:, dp, 0:2, :],
                                start=(i == 0 and dp == 0),
                                stop=(i == 1 and dp == 3),
                                perf_mode=DR)
                    et = etp.tile([128, 2, 256], F8, tag="et")
                    nc.scalar.activation(et[:], st[:], AF.Exp,
                                         bias=biasp[:, j, u:u + 1],
                                         scale=SCALE)
                    ets[u] = et

                def emit_pv(u):
                    et = ets.pop(u)
                    for s in range(2):
                        lhs = et[:, 0:2, s * 128:(s + 1) * 128]
                        for dh in range(2):
                            nc.tensor.matmul(
                                oc[s][:, dh * 512:(dh + 1) * 512], lhs,
                                v8[:, u, 0:2, dh * 512:(dh + 1) * 512],
                                start=(u == 0), stop=False, perf_mode=DR)
                        nc.tensor.matmul(sums[s][:], lhs, ones8[:],
                                         start=(u == 0), stop=False,
                                         perf_mode=DR)

                for u in range(PJn + 2):
                    if u < PJn:
                        emit_qk(u)
                    if u >= 2:
                        emit_pv(u - 2)

                # ---- boundary scores: hi*hi + hi*lo + lo*hi into one tile
                st_b = pst.tile([128, 2, 256], F32, tag="st", name=f"stb{j}")
                terms = [(kb8, qt8), (kb8, qlo), (klo, qt8)]
                n_mm = len(terms) * 2 * 4
                k = 0
                for lt_, rt_ in terms:
                    for i in range(2):
                        for dp in range(4):
                            nc.tensor.matmul(
                                st_b[:, i, :],
                                lt_[:, dp, 0:2, i * 128:(i + 1) * 128],
                                rt_[:, dp, 0:2, :],
                                start=(k == 0), stop=(k == n_mm - 1),
                                perf_mode=DR)
                            k += 1

                # ---- wedge (cross within the boundary block), bf16
                ew = ewp.tile([128, 2, 256], BF16, tag="ew")
                nc.gpsimd.memset(ew[:], 0.0)
                nc.scalar.activation(ew[:, 0, 64:128], st_b[:, 0, 64:128],
                                     AF.Exp, bias=biasb[:, 0:1], scale=SCALE)
                nc.scalar.activation(ew[:, 0, 128:256], st_b[:, 0, 128:256],
                                     AF.Exp, bias=biasb[:, 1:2], scale=SCALE)
                nc.scalar.activation(ew[:, 1, 192:256], st_b[:, 1, 192:256],
                                     AF.Exp, bias=biasb[:, 0:1], scale=SCALE)

                # ---- local diagonal exp -> block-diagonal bf16 lhsT
                lhs_loc = []
                for s in range(2):
                    Lh = lhp.tile([128, 128], BF16, tag="lh")
                    nc.gpsimd.memset(Lh[:], 0.0)
                    nc.scalar.activation(
                        Lh[0:64, 0:64],
                        st_b[0:64, s, s * 128:s * 128 + 64],
                        AF.Exp, bias=biasb[0:64, 1:2], scale=SCALE)
                    nc.scalar.activation(
                        Lh[64:128, 64:128],
                        st_b[64:128, s, s * 128 + 64:s * 128 + 128],
                        AF.Exp, bias=biasb[64:128, 1:2], scale=SCALE)
                    lhs_loc.append(Lh)

                # ---- wedge PV + sums (accumulate into oc/sums, then stop)
                first_oc = (PJn == 0)
                for dh in range(2):
                    nc.tensor.matmul(oc[0][:, dh * 512:(dh + 1) * 512],
                                     ew[:, 0, 0:128],
                                     vb16[:, j, 0, dh * 512:(dh + 1) * 512],
                                     start=first_oc, stop=True)
                nc.tensor.matmul(sums[0][:], ew[:, 0, 0:128], ones16[:],
                                 start=first_oc, stop=True)
                for dh in range(2):
                    nc.tensor.matmul(oc[1][:, dh * 512:(dh + 1) * 512],
                                     ew[:, 0, 128:256],
                                     vb16[:, j, 0, dh * 512:(dh + 1) * 512],
                                     start=first_oc, stop=False)
                for dh in range(2):
                    nc.tensor.matmul(oc[1][64:128, dh * 512:(dh + 1) * 512],
                                     ew[:, 1, 192:256],
                                     vb16[:, j, 1, dh * 512:(dh + 1) * 512],
                                     start=False, stop=True)
                nc.tensor.matmul(sums[1][:], ew[:, 0, 128:256], ones16[:],
                                 start=first_oc, stop=False)
                nc.tensor.matmul(sums[1][64:128, :], ew[:, 1, 192:256],
                                 ones16[:], start=False, stop=True)

                # ---- local PV + sums (one psum bank, two regions)
                sums_l = pst.tile([128, 4], F32, tag="st", name=f"sl{j}")
                Ls = []
                for s in range(2):
                    L = poc.tile([128, D], F32, tag="oc")
                    for dh in range(2):
                        nc.tensor.matmul(L[:, dh * 512:(dh + 1) * 512],
                                         lhs_loc[s],
                                         vb16[:, j, s, dh * 512:(dh + 1) * 512],
                                         start=True, stop=True)
                    nc.tensor.matmul(sums_l[:, 2 * s:2 * s + 2], lhs_loc[s],
                                     ones16[:], start=(s == 0), stop=True)
                    Ls.append(L)

                # ---- normalize, blend, emit
                for s in range(2):
                    scm = vecp.tile([128, 1], F32, tag="v")
                    nc.vector.tensor_scalar_max(scm[:], sums[s][:, 0:1], 1e-30)
                    rc = vecp.tile([128, 1], F32, tag="v")
                    nc.vector.reciprocal(rc[:], scm[:])
                    rc2 = vecp.tile([128, 1], F32, tag="v")
                    nc.vector.tensor_mul(rc2[:], rc[:],
                                         blend[:, j, 2 * s + 1:2 * s + 2])
                    cs = outp.tile([128, D], F32, tag="out")
                    nc.scalar.activation(cs[:], oc[s][:], AF.Copy,
                                         scale=rc2[:])

                    slm = vecp.tile([128, 1], F32, tag="v")
                    nc.vector.tensor_scalar_max(
                        slm[:], sums_l[:, 2 * s:2 * s + 1], 1e-30)
                    rl = vecp.tile([128, 1], F32, tag="v")
                    nc.vector.reciprocal(rl[:], slm[:])
                    rl2 = vecp.tile([128, 1], F32, tag="v")
                    nc.vector.tensor_mul(rl2[:], rl[:],
                                         blend[:, j, 2 * s:2 * s + 1])
                    lt = outp.tile([128, D], F32, tag="out")
                    nc.vector.tensor_scalar_mul(lt[:], Ls[s][:], rl2[:])
                    fin = outp.tile([128, D], F16, tag="outf")
                    nc.gpsimd.tensor_add(fin[:], lt[:], cs[:])
                    row = (2 * j + s) * 128
                    nc.sync.dma_start(out_t[row:row + 128, :], fin[:])
    nc.compile()
    return nc


def _tx(X):
    """[R, 1024] -> [128, 4, 2, R] transposed layout (d = dp*256+i*128+p)."""
    R = X.shape[0]
    return np.ascontiguousarray(
        X.T.reshape(4, 2, 128, R).transpose(2, 0, 1, 3))


def _host_inputs(cfg: Cfg, query, key, value):
    B = query.shape[0]
    shared = {}
    for b in range(B):
        K8 = key[b].astype(FP8)
        V8 = value[b].astype(FP8)
        spre = max(cfg.SPRE, 256)
        kt8 = _tx(K8[:spre])
        nv = max(cfg.maxP, 1)
        v8 = np.ascontiguousarray(
            V8[:nv * 256].reshape(nv, 2, 128, cfg.D).transpose(2, 0, 1, 3))
        shared[b] = (kt8, v8)

    maxP = max(cfg.maxP, 1)
    in_maps = []
    for core in range(cfg.n_cores):
        b = core // cfg.cores_per_batch
        t = core % cfg.cores_per_batch
        groups = [cfg.classes[j][t] for j in range(cfg.n_slot)]
        kt8, v8 = shared[b]

        qslab = np.zeros((128, cfg.n_slot, 4, 4, 2, 256), FP8)
        vb16 = np.zeros((128, cfg.n_slot, 2, cfg.D), BF)
        biasp = np.zeros((128, cfg.n_slot, maxP), np.float32)
        blend = np.zeros((128, cfg.n_slot, 4), np.float32)
        for j, g in enumerate(groups):
            Qg = query[b, g * 256:(g + 1) * 256]
            Kg = key[b, g * 256:(g + 1) * 256]
            Qh = Qg.astype(FP8)
            Kh = Kg.astype(FP8)
            qslab[:, j, 0] = _tx(Qh)
            qslab[:, j, 1] = _tx((Qg - Qh.astype(np.float32)).astype(FP8))
            qslab[:, j, 2] = _tx(Kh)
            qslab[:, j, 3] = _tx((Kg - Kh.astype(np.float32)).astype(FP8))
            for i in range(2):
                vb16[:, j, i, :] = value[b, g * 256 + i * 128:
                                         g * 256 + (i + 1) * 128].astype(BF)
            biasp[:, j, :] = NEG
            biasp[:, j, :g] = SHIFT
            for s in range(2):
                for half in range(2):
                    c = 2 * s + half
                    sl = slice(half * 64, half * 64 + 64)
                    first = (g == 0 and c == 0)
                    blend[sl, j, 2 * s] = 1.0 if first else 0.9
                    blend[sl, j, 2 * s + 1] = 0.0 if first else 0.1
        in_maps.append({
            "kt8_in": kt8,
            "v8_in": v8,
            "qslab_in": qslab,
            "vb16_in": vb16,
            "biasp_in": biasp,
            "blend_in": blend,
        })
    return in_maps


def _scatter_output(cfg: Cfg, results, B):
    out = np.empty((B, cfg.S, cfg.D), np.float32)
    for core in range(cfg.n_cores):
        b = core // cfg.cores_per_batch
        t = core % cfg.cores_per_batch
        oc = results[core]["out_core"].astype(np.float32)
        for j in range(cfg.n_slot):
            g = cfg.classes[j][t]
            out[b, g * 256:(g + 1) * 256] = oc[j * 256:(j + 1) * 256]
    return out


_nc_cache = {}


def run(cfg: Cfg, query, key, value, trace=False, trace_kwargs=None):
    ck = cfg.S
    if ck not in _nc_cache:
        _nc_cache[ck] = build_nc(cfg)
    nc = _nc_cache[ck]
    in_maps = _host_inputs(cfg, query, key, value)
    kw = {}
    if trace:
        kw = dict(trace=True, trace_cores=list(range(cfg.n_cores)),
                  **(trace_kwargs or {}))
    res = run_bass_kernel_spmd(nc, in_maps, core_ids=list(range(cfg.n_cores)),
                               **kw)
    out = _scatter_output(cfg, res.results, query.shape[0])
    return out, res


def kernel(query, key, value):
    query = np.asarray(query, np.float32)
    key = np.asarray(key, np.float32)
    value = np.asarray(value, np.float32)
    out, _ = run(FULL, query, key, value)
    return out
